# revision 16
# baseline (speedup 1.0000x reference)
"""Trainium2 Bass kernel for nn_CausalSelfAttention (sliding-window GQA attention).

Sharding (8 cores): data-parallel over batch B=2 (cores 0-3 -> b=0, 4-7 -> b=1),
tensor-parallel over heads within each batch: each core handles 4 query heads
(one KV head, GQA rep=4). c_q/c_k/c_v column-parallel, c_proj row-parallel;
the row-parallel reduce is done on the host during unsharding.

Per-core pipeline (single NeuronCore, Tile-scheduled):
  A: fused QKV+gate projection (fp32r matmuls), RoPE + per-head rmsnorm in
     natural layout (DVE/ACT; rmsnorm commutes with RoPE since rotations are
     norm-preserving), value-embedding gating, PE transposes to build
     q^T / k^T with the head dim on partitions.
  B: sliding-window attention per 256-query supertile: scoresT matmuls
     (2 heads row-packed in the PE array via tile_position), one Exp per
     (head, supertile) on ACT, triangular masking via gpsimd affine_select,
     AV matmuls with a ones-column producing softmax denominators for free.
  C: denominator transpose via a small DRAM round-trip + partition-broadcast
     DMA, normalization muls, then the output projection (fp32r) and DMA out.
"""

import sys

for _p in ("/opt/trn_rl_repo", "/root/.axon_site/_ro/trn_rl_repo"):
    if _p not in sys.path:
        sys.path.insert(0, _p)

import numpy as np
import ml_dtypes

import concourse.bass as bass
import concourse.mybir as mybir
import concourse.tile as tile
from concourse import bacc
from concourse.bass_utils import run_bass_kernel_spmd

dt = mybir.dt
AF = mybir.ActivationFunctionType
OP = mybir.AluOpType

B, S, D = 2, 2048, 1024
H, KVH = 16, 4
HD = 64          # head dim
W = 512          # window size
NCORES = 8
NT = S // 128    # 16 s-tiles
NT2 = NT // 2    # 8 pair-tiles
NU = S // 256    # 8 query supertiles
A_SCALE = 1.2 / np.sqrt(8.0)       # folded into cos/sin tables
S_PRIME = float(1.0 / (64.0 * A_SCALE * A_SCALE))  # rescales ss -> mean(rot^2)
EPS = 1e-6

ATTN_DT = dt.bfloat16  # dtype of q^T/k^T/v/expT and the attention matmuls


def bcast_dim(ap, pos, count):
    """Insert a stride-0 broadcast dim into an AP at position `pos`."""
    dims = [list(d) for d in ap.ap]
    dims.insert(pos, [0, count])
    return bass.AP(tensor=ap.tensor, offset=ap.offset, ap=dims)


def _segments(u):
    """Score segments for query supertile u (queries [256u, 256u+256)).

    (kt, col, n, q_off, mask): kt = key tile (keys [128kt, 128kt+128)),
    col = column offset in the sc psum tensor, n = #query columns,
    q_off = query offset within the supertile, mask in
    {None, ("LT", blk), ("UT", blk)} marking a 128-col block at local
    column blk that needs the lower/upper-triangular window mask.
    The first entry spans q_off 0..256 so AV can issue it start=True.
    """
    Q0 = 256 * u
    segs = []
    col = 0
    for d, n, q_off, mask in (
        (256, 256, 0, None),
        (384, 256, 0, ("LT", 128)),
        (128, 256, 0, None),
        (0, 256, 0, ("UT", 0)),
        (512, 128, 0, ("LT", 0)),
        (-128, 128, 128, ("UT", 0)),
    ):
        K0 = Q0 - d
        if K0 < 0:
            continue
        segs.append((K0 // 128, col, n, q_off, mask))
        col += n
    return segs


def _cols(u):
    return sum(n for _, _, n, _, _ in _segments(u))


def build_nc():
    nc = bacc.Bacc("TRN2", target_bir_lowering=False, debug=False,
                   num_devices=NCORES)

    xT_d = nc.dram_tensor("xT", [D, S], dt.float32r, kind="ExternalInput").ap()
    w_d = nc.dram_tensor("W", [D, 386], dt.float32r, kind="ExternalInput").ap()
    wo_d = nc.dram_tensor("Wo", [256, D], dt.float32r, kind="ExternalInput").ap()
    ve3_d = nc.dram_tensor("ve3", [S, HD], dt.bfloat16, kind="ExternalInput").ap()
    cos_d = nc.dram_tensor("cosF", [S, HD], dt.bfloat16, kind="ExternalInput").ap()
    sin_d = nc.dram_tensor("sinFs", [S, HD], dt.bfloat16, kind="ExternalInput").ap()
    id_d = nc.dram_tensor("ident", [128, 128], dt.bfloat16, kind="ExternalInput").ap()
    out_d = nc.dram_tensor("out", [S, D], dt.float32, kind="ExternalOutput").ap()

    with tile.TileContext(nc) as tc:
        with (
            tc.tile_pool(name="res", bufs=1) as res,     # long-lived SBUF
            tc.tile_pool(name="pa", bufs=2) as pa,       # phase-A scratch
            tc.tile_pool(name="pb", bufs=3) as pb,       # phase-B scratch
            tc.tile_pool(name="pbx", bufs=4) as pbx,     # normalize scratch
            tc.tile_pool(name="pc", bufs=2) as pc,       # phase-C scratch
            tc.tile_pool(name="dram", bufs=1, space="DRAM") as dram,
        ):
            # ---- resident inputs (emitted in consumption order) ------------
            xT_sb = [res.tile([128, S], dt.float32r, tag=f"xT{k}", name=f"xT{k}") for k in range(8)]
            w_sb = [res.tile([128, 386], dt.float32r, tag=f"W{k}", name=f"W{k}") for k in range(8)]
            for k in range(8):
                nc.sync.dma_start(out=w_sb[k][:], in_=w_d[128 * k:128 * (k + 1), :])
                nc.sync.dma_start(out=xT_sb[k][:], in_=xT_d[128 * k:128 * (k + 1), :])
            id_sb = res.tile([128, 128], dt.bfloat16, tag="ident")
            nc.sync.dma_start(out=id_sb[:], in_=id_d)
            wo_sb = [res.tile([128, D], dt.float32r, tag=f"Wo{p}", name=f"Wo{p}") for p in range(2)]

            # [S, HD] tables -> [128, NT, HD] (s = t*128 + p)
            def load_tiled(name, dram_ap):
                t = res.tile([128, NT, HD], dt.bfloat16, tag=name)
                src = bass.AP(tensor=dram_ap.tensor, offset=0,
                              ap=[[HD, 128], [128 * HD, NT], [1, HD]])
                nc.sync.dma_start(out=t[:], in_=src)
                return t

            ve3_sb = load_tiled("ve3", ve3_d)
            cos_sb = load_tiled("cosF", cos_d)
            sin_sb = load_tiled("sinFs", sin_d)
            for p in range(2):
                nc.sync.dma_start(out=wo_sb[p][:], in_=wo_d[128 * p:128 * (p + 1), :])

            # ---- long-lived intermediates ----------------------------------
            # qT: cols [0,S) pair0 (h0 rows 0-63, h1 rows 64-127), [S,2S) pair1
            qT_sb = res.tile([128, 2 * S], ATTN_DT, tag="qT")
            kT_sb = res.tile([128, S], ATTN_DT, tag="kT")  # rows 64-127 duplicate
            v_sb = res.tile([128, NT * (HD + 1)], ATTN_DT, tag="v")
            yraw = [res.tile([65, S], dt.float32, tag=f"yraw{h}", name=f"yraw{h}") for h in range(4)]
            pairT = [res.tile([128, S], dt.float32r, tag=f"pairT{p}", name=f"pairT{p}")
                     for p in range(2)]

            ones64f = res.tile([1, 64], dt.float32, tag="ones64f")
            nc.vector.memset(ones64f[:], 1.0)
            ones64 = res.tile([1, 64], dt.float32r, tag="ones64")
            nc.vector.tensor_copy(out=ones64[:], in_=ones64f[:])
            cst = res.tile([128, 1], dt.int32, tag="cmagic")
            nc.vector.memset(cst[:], 0x5F3759DF)
            c15 = res.tile([128, 1], dt.float32, tag="c15")
            nc.vector.memset(c15[:], 1.5)

            v_view = v_sb[:].rearrange("p (t c) -> p t c", c=HD + 1)
            nc.vector.memset(v_view[:, :, HD:HD + 1], 1.0)

            # ============ Phase A: projection + postprocessing ==============
            with (
                tc.tile_pool(name="pp_ps", bufs=2, space="PSUM") as pp_ps,
                tc.tile_pool(name="tp_ps", bufs=2, space="PSUM") as tp_ps,
            ):
                for t2 in range(NT2):
                    pp = pp_ps.tile([128, 1024], dt.float32, tag="pp")
                    ppv = pp[:].rearrange("p (t c) -> p t c", c=512)
                    for tt in range(2):
                        s_tile = 2 * t2 + tt
                        for k in range(8):
                            nc.tensor.matmul(
                                ppv[:, tt, 0:386],
                                xT_sb[k][:, 128 * s_tile:128 * (s_tile + 1)],
                                w_sb[k][:],
                                start=(k == 0), stop=(k == 7),
                            )

                    # gate = 1/(1+exp(-z))
                    gate = pa.tile([128, 2], dt.float32, tag="gate")
                    nc.scalar.activation(
                        out=gate[:].rearrange("p (t c) -> p t c", c=1),
                        in_=ppv[:, :, 384:385], func=AF.Exp, scale=-1.0,
                    )
                    nc.vector.tensor_scalar_add(gate[:], gate[:], 1.0)
                    nc.vector.reciprocal(out=gate[:], in_=gate[:])

                    # v = ve3*gate + v_proj
                    for tt in range(2):
                        s_tile = 2 * t2 + tt
                        nc.vector.scalar_tensor_tensor(
                            out=v_view[:, s_tile, 0:HD],
                            in0=ve3_sb[:, s_tile, :],
                            scalar=gate[:, tt:tt + 1],
                            in1=ppv[:, tt, 320:384],
                            op0=OP.mult, op1=OP.add,
                        )

                    # RoPE on q,k (cols 0:320 per sub-tile), blocks of 64
                    qk4 = ppv[:, :, 0:320].rearrange("p t (b x) -> p t b x", x=64)
                    sw = pa.tile([128, 640], ATTN_DT, tag="sw")
                    sw4 = sw[:].rearrange("p (t b x) -> p t b x", t=2, x=64)
                    nc.scalar.copy(out=sw4[:, :, :, 0:32],
                                   in_=qk4[:, :, :, 32:64])
                    nc.scalar.copy(out=sw4[:, :, :, 32:64],
                                   in_=qk4[:, :, :, 0:32])

                    cosb = bcast_dim(cos_sb[:, 2 * t2:2 * t2 + 2, :], 2, 5)
                    sinb = bcast_dim(sin_sb[:, 2 * t2:2 * t2 + 2, :], 2, 5)

                    rot = pa.tile([128, 640], ATTN_DT, tag="rot")
                    rot4 = rot[:].rearrange("p (t b x) -> p t b x", t=2, x=64)
                    nc.vector.tensor_tensor(out=rot4, in0=qk4, in1=cosb, op=OP.mult)
                    rot2 = pa.tile([128, 640], ATTN_DT, tag="rot2")
                    rot24 = rot2[:].rearrange("p (t b x) -> p t b x", t=2, x=64)
                    nc.gpsimd.tensor_tensor(out=rot24, in0=sw4, in1=sinb, op=OP.mult)
                    nc.vector.tensor_tensor(out=rot4, in0=rot4, in1=rot24, op=OP.add)

                    # rmsnorm: r = rsqrt(S_PRIME*ss + EPS), fast-inv-sqrt on DVE
                    sq = pa.tile([128, 640], ATTN_DT, tag="sqv")
                    sq4 = sq[:].rearrange("p (t b x) -> p t b x", t=2, x=64)
                    nc.scalar.square(out=sq4, in_=rot4)
                    ss = pa.tile([128, 10], dt.float32, tag="ss")
                    nc.vector.tensor_reduce(
                        out=ss[:].rearrange("p (t b x) -> p t b x", t=2, x=1),
                        in_=sq4, op=OP.add, axis=mybir.AxisListType.X)
                    wv = pa.tile([128, 10], dt.float32, tag="wv")
                    nc.vector.tensor_scalar(
                        out=wv[:], in0=ss[:], scalar1=S_PRIME, scalar2=EPS,
                        op0=OP.mult, op1=OP.add)
                    yi = pa.tile([128, 10], dt.int32, tag="yi")
                    nc.vector.tensor_scalar(
                        out=yi[:], in0=wv[:].bitcast(dt.int32), scalar1=1,
                        scalar2=None, op0=OP.logical_shift_right)
                    nc.vector.tensor_tensor(
                        out=yi[:], in0=cst[:].to_broadcast((128, 10)), in1=yi[:],
                        op=OP.subtract)
                    y0 = yi[:].bitcast(dt.float32)
                    t1 = pa.tile([128, 10], dt.float32, tag="t1")
                    for _ in range(2):
                        nc.vector.tensor_tensor(out=t1[:], in0=y0, in1=y0, op=OP.mult)
                        nc.vector.tensor_tensor(out=t1[:], in0=t1[:], in1=wv[:],
                                                op=OP.mult)
                        nc.vector.scalar_tensor_tensor(
                            out=t1[:], in0=t1[:], scalar=-0.5,
                            in1=c15[:].to_broadcast((128, 10)),
                            op0=OP.mult, op1=OP.add)
                        nc.vector.tensor_tensor(out=y0, in0=y0, in1=t1[:], op=OP.mult)

                    qk_hat = pa.tile([128, 640], ATTN_DT, tag="qk_hat")
                    qk_hat4 = qk_hat[:].rearrange("p (t b x) -> p t b x", t=2, x=64)
                    rb = bcast_dim(
                        yi[:].bitcast(dt.float32).rearrange("p (t b) -> p t b", t=2),
                        3, 64)
                    nc.vector.tensor_tensor(out=qk_hat4, in0=rot4, in1=rb, op=OP.mult)

                    # transposes -> qT / kT
                    for tt in range(2):
                        s_tile = 2 * t2 + tt
                        tp = tp_ps.tile([128, 384], ATTN_DT, tag="tp")
                        for ft in range(2):
                            nc.tensor.transpose(
                                tp[:, 128 * ft:128 * (ft + 1)],
                                qk_hat[:, tt * 320 + 128 * ft:
                                       tt * 320 + 128 * (ft + 1)],
                                id_sb[:])
                        nc.tensor.transpose(
                            tp[0:64, 256:384],
                            qk_hat[:, tt * 320 + 256: tt * 320 + 320], id_sb[:])
                        nc.tensor.transpose(
                            tp[64:128, 256:384],
                            qk_hat[:, tt * 320 + 256: tt * 320 + 320], id_sb[:])
                        qdst = bass.AP(
                            tensor=qT_sb.tensor, offset=128 * s_tile,
                            ap=[[2 * S, 128], [S, 2], [1, 128]])
                        nc.vector.tensor_copy(
                            out=qdst,
                            in_=tp[:, 0:256].rearrange("p (b x) -> p b x", b=2))
                        nc.scalar.copy(
                            out=kT_sb[:, 128 * s_tile:128 * (s_tile + 1)],
                            in_=tp[:, 256:384])

            # ============ Phase B: attention + normalize + out proj =========
            with (
                tc.tile_pool(name="sc_ps", bufs=2, space="PSUM") as sc_ps,
                tc.tile_pool(name="acc_ps", bufs=2, space="PSUM") as acc_ps,
            ):
                def emit_attention(u):
                    segs = _segments(u)
                    ncols = _cols(u)
                    for pair in range(2):
                        exps = []
                        for hh in range(2):
                            sc = sc_ps.tile([128, 1280], dt.float32, tag="sc",
                                            name=f"sc_{u}_{pair}_{hh}")
                            for kt, col, n, q_off, _m in segs:
                                nc.tensor.matmul(
                                    sc[:, col:col + n],
                                    kT_sb[64 * hh:64 * (hh + 1),
                                          128 * kt:128 * (kt + 1)],
                                    qT_sb[64 * hh:64 * (hh + 1),
                                          S * pair + 256 * u + q_off:
                                          S * pair + 256 * u + q_off + n],
                                    start=True, stop=True,
                                    tile_position=(64 * hh, 0),
                                )
                            ex = pb.tile([128, 1280], ATTN_DT, tag="ex",
                                         name=f"ex_{u}_{pair}_{hh}")
                            nc.scalar.activation(out=ex[:, 0:ncols],
                                                 in_=sc[:, 0:ncols], func=AF.Exp)
                            for kt, col, n, q_off, m in segs:
                                if m is None:
                                    continue
                                kind, blk = m
                                blk_ap = ex[:, col + blk: col + blk + 128]
                                if kind == "LT":  # keep j < i
                                    nc.gpsimd.affine_select(
                                        out=blk_ap, in_=blk_ap,
                                        compare_op=OP.is_gt, fill=0.0, base=0,
                                        pattern=[[-1, 128]], channel_multiplier=1)
                                else:             # UT: keep j >= i
                                    nc.gpsimd.affine_select(
                                        out=blk_ap, in_=blk_ap,
                                        compare_op=OP.is_ge, fill=0.0, base=0,
                                        pattern=[[1, 128]], channel_multiplier=-1)
                            exps.append(ex)

                        for hh in range(2):
                            h = 2 * pair + hh
                            ex = exps[hh]
                            yt = acc_ps.tile([65, 256], dt.float32, tag="acc",
                                             name=f"yt_{u}_{h}")
                            last = len(segs) - 1
                            for i, (kt, col, n, q_off, _m) in enumerate(segs):
                                nc.tensor.matmul(
                                    yt[:, q_off:q_off + n],
                                    v_view[:, kt, :],
                                    ex[:, col:col + n],
                                    start=(i == 0), stop=(i == last),
                                )
                            nc.vector.tensor_copy(
                                out=yraw[h][:, 256 * u:256 * (u + 1)],
                                in_=yt[0:65, :])

                def emit_gather(u):
                    den32 = pbx.tile([32, 32], dt.float32, tag="den32",
                                     name=f"den32_{u}")
                    for h in range(4):
                        nc.sync.dma_start(
                            out=den32[8 * h:8 * (h + 1), :],
                            in_=yraw[h][64:65, 256 * u:256 * (u + 1)])
                    den32r = pbx.tile([32, 32], dt.float32r, tag="den32r",
                                      name=f"den32r_{u}")
                    with nc.allow_low_precision(reason="f32r bits == f32"):
                        nc.vector.reciprocal(out=den32r[:], in_=den32[:])
                    rrs = []
                    for h in range(4):
                        rr_h = pbx.tile([1, 256], dt.float32r, tag=f"rr{h}",
                                        name=f"rr_{u}_{h}")
                        nc.sync.dma_start(
                            out=rr_h[:],
                            in_=den32r[8 * h:8 * (h + 1), :])
                        rrs.append(rr_h)
                    setattr(emit_gather, f"rr_{u}", rrs)

                def emit_tail(u):
                    rrs = getattr(emit_gather, f"rr_{u}")
                    for h in range(4):
                        pair, hh = divmod(h, 2)
                        rb_ps = acc_ps.tile([64, 256], dt.float32, tag="acc",
                                            name=f"rb_{u}_{h}")
                        nc.tensor.matmul(rb_ps[:], ones64[:], rrs[h][:],
                                         start=True, stop=True)
                        if hh == 0:
                            nc.vector.tensor_tensor(
                                out=pairT[pair][0:64, 256 * u:256 * (u + 1)],
                                in0=yraw[h][0:64, 256 * u:256 * (u + 1)],
                                in1=rb_ps[:], op=OP.mult)
                        else:
                            stg = pbx.tile([64, 256], dt.float32r, tag="stg",
                                           name=f"stg_{u}_{h}")
                            nc.vector.tensor_tensor(
                                out=stg[:],
                                in0=yraw[h][0:64, 256 * u:256 * (u + 1)],
                                in1=rb_ps[:], op=OP.mult)
                            nc.sync.dma_start(
                                out=pairT[pair][64:128, 256 * u:256 * (u + 1)],
                                in_=stg[:])
                    for t in (2 * u, 2 * u + 1):
                        ot = pc.tile([128, 1024], dt.float32, tag="ot",
                                     name=f"ot_{t}")
                        for n in range(2):
                            op_ps = acc_ps.tile([128, 512], dt.float32,
                                                tag="acc", name=f"op_{t}_{n}")
                            for pair in range(2):
                                nc.tensor.matmul(
                                    op_ps[:],
                                    pairT[pair][:, 128 * t:128 * (t + 1)],
                                    wo_sb[pair][:, 512 * n:512 * (n + 1)],
                                    start=(pair == 0), stop=(pair == 1),
                                )
                            if (t + n) % 2 == 0:
                                nc.vector.tensor_copy(
                                    out=ot[:, 512 * n:512 * (n + 1)], in_=op_ps[:])
                            else:
                                nc.scalar.copy(
                                    out=ot[:, 512 * n:512 * (n + 1)], in_=op_ps[:])
                        nc.sync.dma_start(out=out_d[128 * t:128 * (t + 1), :],
                                          in_=ot[:])

                # software-pipelined emission: tail lags attention by two u
                for u in range(NU):
                    emit_attention(u)
                    emit_gather(u)
                    if u >= 2:
                        emit_tail(u - 2)
                emit_tail(NU - 2)
                emit_tail(NU - 1)

    nc.finalize()
    return nc


_CACHED_NC = None


def _get_nc():
    global _CACHED_NC
    if _CACHED_NC is None:
        _CACHED_NC = build_nc()
    return _CACHED_NC


def _prep_core(x, ve, cosf, sinf, Wq, Wk, Wv, Wo, Wg, b, g):
    xT = np.ascontiguousarray(x[b].T.astype(np.float32))
    wg_pad = np.zeros((D, 2), np.float32)
    wg_pad[:Wg.shape[0], 0] = Wg[:, g]
    w_all = np.concatenate(
        [Wq[:, 256 * g:256 * (g + 1)], Wk[:, 64 * g:64 * (g + 1)],
         Wv[:, 64 * g:64 * (g + 1)], wg_pad], axis=1).astype(np.float32)
    wo_g = np.ascontiguousarray(Wo[256 * g:256 * (g + 1), :].astype(np.float32))
    ve3 = (3.0 * ve[b, :, 64 * g:64 * (g + 1)]).astype(ml_dtypes.bfloat16)
    return {
        "xT": xT, "W": w_all, "Wo": wo_g, "ve3": ve3,
        "cosF": cosf, "sinFs": sinf,
        "ident": np.eye(128, dtype=ml_dtypes.bfloat16),
    }


def kernel(x, ve, cos, sin, Wq, Wk, Wv, Wo, Wg, window_size):
    assert int(window_size) == W
    x = np.asarray(x, np.float32)
    ve = np.asarray(ve, np.float32)
    cos2 = np.asarray(cos, np.float32).reshape(S, 32)
    sin2 = np.asarray(sin, np.float32).reshape(S, 32)
    a = np.float32(A_SCALE)
    cosf = (a * np.concatenate([cos2, cos2], axis=1)).astype(ml_dtypes.bfloat16)
    sinf = (a * np.concatenate([sin2, -sin2], axis=1)).astype(ml_dtypes.bfloat16)

    nc = _get_nc()
    in_maps = [
        _prep_core(x, ve, cosf, sinf,
                   np.asarray(Wq, np.float32), np.asarray(Wk, np.float32),
                   np.asarray(Wv, np.float32), np.asarray(Wo, np.float32),
                   np.asarray(Wg, np.float32), c // 4, c % 4)
        for c in range(NCORES)
    ]
    res = run_bass_kernel_spmd(nc, in_maps, core_ids=list(range(NCORES)))
    out = np.zeros((B, S, D), np.float32)
    for c in range(NCORES):
        out[c // 4] += res.results[c]["out"]
    return out


# revision 17
# speedup vs baseline: 1.2020x; 1.2020x over previous
"""Trainium2 Bass kernel for nn_CausalSelfAttention (sliding-window GQA attention).

Sharding (8 cores): data-parallel over batch B=2 (cores 0-3 -> b=0, 4-7 -> b=1),
tensor-parallel over heads within each batch: each core handles 4 query heads
(one KV head, GQA rep=4). c_q/c_k/c_v column-parallel, c_proj row-parallel;
the row-parallel reduce is done on the host during unsharding.

Per-core pipeline (single NeuronCore, Tile-scheduled):
  A: fused QKV+gate projection (fp32r matmuls), RoPE + per-head rmsnorm in
     natural layout (DVE/ACT; rmsnorm commutes with RoPE since rotations are
     norm-preserving), value-embedding gating, PE transposes to build
     q^T / k^T with the head dim on partitions.
  B: sliding-window attention per 256-query supertile: scoresT matmuls
     (2 heads row-packed in the PE array via tile_position), one Exp per
     (head, supertile) on ACT, triangular masking via gpsimd affine_select,
     AV matmuls with a ones-column producing softmax denominators for free.
  C: denominator transpose via a small DRAM round-trip + partition-broadcast
     DMA, normalization muls, then the output projection (fp32r) and DMA out.
"""

import sys

for _p in ("/opt/trn_rl_repo", "/root/.axon_site/_ro/trn_rl_repo"):
    if _p not in sys.path:
        sys.path.insert(0, _p)

import numpy as np
import ml_dtypes

import concourse.bass as bass
import concourse.mybir as mybir
import concourse.tile as tile
from concourse import bacc
from concourse.bass_utils import run_bass_kernel_spmd

dt = mybir.dt
AF = mybir.ActivationFunctionType
OP = mybir.AluOpType

B, S, D = 2, 2048, 1024
H, KVH = 16, 4
HD = 64          # head dim
W = 512          # window size
NCORES = 8
NT = S // 128    # 16 s-tiles
NT2 = NT // 2    # 8 pair-tiles
NU = S // 256    # 8 query supertiles
A_SCALE = 1.2 / np.sqrt(8.0)       # folded into cos/sin tables
S_PRIME = float(1.0 / (64.0 * A_SCALE * A_SCALE))  # rescales ss -> mean(rot^2)
EPS = 1e-6

ATTN_DT = dt.bfloat16  # dtype of q^T/k^T/v/expT and the attention matmuls


def bcast_dim(ap, pos, count):
    """Insert a stride-0 broadcast dim into an AP at position `pos`."""
    dims = [list(d) for d in ap.ap]
    dims.insert(pos, [0, count])
    return bass.AP(tensor=ap.tensor, offset=ap.offset, ap=dims)


def _segments(u):
    """Score segments for query supertile u (queries [256u, 256u+256)).

    (kt, col, n, q_off, mask): kt = key tile (keys [128kt, 128kt+128)),
    col = column offset in the sc psum tensor, n = #query columns,
    q_off = query offset within the supertile, mask in
    {None, ("LT", blk), ("UT", blk)} marking a 128-col block at local
    column blk that needs the lower/upper-triangular window mask.
    The first entry spans q_off 0..256 so AV can issue it start=True.
    """
    Q0 = 256 * u
    segs = []
    col = 0
    for d, n, q_off, mask in (
        (256, 256, 0, None),
        (384, 256, 0, ("LT", 128)),
        (128, 256, 0, None),
        (0, 256, 0, ("UT", 0)),
        (512, 128, 0, ("LT", 0)),
        (-128, 128, 128, ("UT", 0)),
    ):
        K0 = Q0 - d
        if K0 < 0:
            continue
        segs.append((K0 // 128, col, n, q_off, mask))
        col += n
    return segs


def _cols(u):
    return sum(n for _, _, n, _, _ in _segments(u))


def build_nc():
    nc = bacc.Bacc("TRN2", target_bir_lowering=False, debug=False,
                   num_devices=NCORES)

    xT_d = nc.dram_tensor("xT", [D, S], dt.float32r, kind="ExternalInput").ap()
    w_d = nc.dram_tensor("W", [D, 386], dt.float32r, kind="ExternalInput").ap()
    wo_d = nc.dram_tensor("Wo", [256, D], dt.float32r, kind="ExternalInput").ap()
    ve3_d = nc.dram_tensor("ve3", [S, HD], dt.bfloat16, kind="ExternalInput").ap()
    cos_d = nc.dram_tensor("cosF", [S, HD], dt.bfloat16, kind="ExternalInput").ap()
    sin_d = nc.dram_tensor("sinFs", [S, HD], dt.bfloat16, kind="ExternalInput").ap()
    id_d = nc.dram_tensor("ident", [128, 128], dt.bfloat16, kind="ExternalInput").ap()
    out_d = nc.dram_tensor("out", [S, D], dt.float32, kind="ExternalOutput").ap()

    with tile.TileContext(nc) as tc:
        with (
            tc.tile_pool(name="res", bufs=1) as res,     # long-lived SBUF
            tc.tile_pool(name="pa", bufs=2) as pa,       # phase-A scratch
            tc.tile_pool(name="pb", bufs=3) as pb,       # phase-B scratch
            tc.tile_pool(name="pbx", bufs=4) as pbx,     # normalize scratch
            tc.tile_pool(name="pc", bufs=2) as pc,       # phase-C scratch
            tc.tile_pool(name="dram", bufs=1, space="DRAM") as dram,
        ):
            # ---- resident inputs (emitted in consumption order) ------------
            xT_sb = [res.tile([128, S], dt.float32r, tag=f"xT{k}", name=f"xT{k}") for k in range(8)]
            w_sb = [res.tile([128, 386], dt.float32r, tag=f"W{k}", name=f"W{k}") for k in range(8)]
            for k in range(8):
                nc.sync.dma_start(out=w_sb[k][:], in_=w_d[128 * k:128 * (k + 1), :])
                nc.sync.dma_start(out=xT_sb[k][:], in_=xT_d[128 * k:128 * (k + 1), :])
            id_sb = res.tile([128, 128], dt.bfloat16, tag="ident")
            nc.sync.dma_start(out=id_sb[:], in_=id_d)
            wo_sb = [res.tile([128, D], dt.float32r, tag=f"Wo{p}", name=f"Wo{p}") for p in range(2)]

            # [S, HD] tables -> [128, NT, HD] (s = t*128 + p)
            def load_tiled(name, dram_ap):
                t = res.tile([128, NT, HD], dt.bfloat16, tag=name)
                src = bass.AP(tensor=dram_ap.tensor, offset=0,
                              ap=[[HD, 128], [128 * HD, NT], [1, HD]])
                nc.sync.dma_start(out=t[:], in_=src)
                return t

            ve3_sb = load_tiled("ve3", ve3_d)
            cos_sb = load_tiled("cosF", cos_d)
            sin_sb = load_tiled("sinFs", sin_d)
            for p in range(2):
                nc.sync.dma_start(out=wo_sb[p][:], in_=wo_d[128 * p:128 * (p + 1), :])

            # ---- long-lived intermediates ----------------------------------
            # qT: cols [0,S) pair0 (h0 rows 0-63, h1 rows 64-127), [S,2S) pair1
            qT_sb = res.tile([128, 2 * S], ATTN_DT, tag="qT")
            kT_sb = res.tile([128, S], ATTN_DT, tag="kT")  # rows 64-127 duplicate
            v_sb = res.tile([128, NT * (HD + 1)], ATTN_DT, tag="v")
            yraw = [res.tile([65, S], dt.float32, tag=f"yraw{h}", name=f"yraw{h}") for h in range(4)]
            pairT = [res.tile([128, S], dt.float32r, tag=f"pairT{p}", name=f"pairT{p}")
                     for p in range(2)]

            ones64f = res.tile([1, 64], dt.float32, tag="ones64f")
            nc.vector.memset(ones64f[:], 1.0)
            ones64 = res.tile([1, 64], dt.float32r, tag="ones64")
            nc.vector.tensor_copy(out=ones64[:], in_=ones64f[:])
            cst = res.tile([128, 1], dt.int32, tag="cmagic")
            nc.vector.memset(cst[:], 0x5F3759DF)
            c15 = res.tile([128, 1], dt.float32, tag="c15")
            nc.vector.memset(c15[:], 1.5)

            v_view = v_sb[:].rearrange("p (t c) -> p t c", c=HD + 1)
            nc.vector.memset(v_view[:, :, HD:HD + 1], 1.0)

            # ============ Phase A: projection + postprocessing ==============
            with (
                tc.tile_pool(name="pp_ps", bufs=2, space="PSUM") as pp_ps,
                tc.tile_pool(name="tp_ps", bufs=2, space="PSUM") as tp_ps,
            ):
                for t2 in range(NT2):
                    pp = pp_ps.tile([128, 1024], dt.float32, tag="pp")
                    ppv = pp[:].rearrange("p (t c) -> p t c", c=512)
                    for tt in range(2):
                        s_tile = 2 * t2 + tt
                        for k in range(8):
                            nc.tensor.matmul(
                                ppv[:, tt, 0:386],
                                xT_sb[k][:, 128 * s_tile:128 * (s_tile + 1)],
                                w_sb[k][:],
                                start=(k == 0), stop=(k == 7),
                            )

                    # gate = 1/(1+exp(-z))
                    gate = pa.tile([128, 2], dt.float32, tag="gate")
                    nc.scalar.activation(
                        out=gate[:].rearrange("p (t c) -> p t c", c=1),
                        in_=ppv[:, :, 384:385], func=AF.Exp, scale=-1.0,
                    )
                    nc.vector.tensor_scalar_add(gate[:], gate[:], 1.0)
                    nc.vector.reciprocal(out=gate[:], in_=gate[:])

                    # v = ve3*gate + v_proj
                    for tt in range(2):
                        s_tile = 2 * t2 + tt
                        nc.vector.scalar_tensor_tensor(
                            out=v_view[:, s_tile, 0:HD],
                            in0=ve3_sb[:, s_tile, :],
                            scalar=gate[:, tt:tt + 1],
                            in1=ppv[:, tt, 320:384],
                            op0=OP.mult, op1=OP.add,
                        )

                    # RoPE on q,k (cols 0:320 per sub-tile), blocks of 64
                    qk4 = ppv[:, :, 0:320].rearrange("p t (b x) -> p t b x", x=64)
                    sw = pa.tile([128, 640], ATTN_DT, tag="sw")
                    sw4 = sw[:].rearrange("p (t b x) -> p t b x", t=2, x=64)
                    nc.scalar.copy(out=sw4[:, :, :, 0:32],
                                   in_=qk4[:, :, :, 32:64])
                    nc.scalar.copy(out=sw4[:, :, :, 32:64],
                                   in_=qk4[:, :, :, 0:32])

                    cosb = bcast_dim(cos_sb[:, 2 * t2:2 * t2 + 2, :], 2, 5)
                    sinb = bcast_dim(sin_sb[:, 2 * t2:2 * t2 + 2, :], 2, 5)

                    rot = pa.tile([128, 640], ATTN_DT, tag="rot")
                    rot4 = rot[:].rearrange("p (t b x) -> p t b x", t=2, x=64)
                    nc.vector.tensor_tensor(out=rot4, in0=qk4, in1=cosb, op=OP.mult)
                    rot2 = pa.tile([128, 640], ATTN_DT, tag="rot2")
                    rot24 = rot2[:].rearrange("p (t b x) -> p t b x", t=2, x=64)
                    nc.gpsimd.tensor_tensor(out=rot24, in0=sw4, in1=sinb, op=OP.mult)
                    nc.vector.tensor_tensor(out=rot4, in0=rot4, in1=rot24, op=OP.add)

                    # rmsnorm: r = rsqrt(S_PRIME*ss + EPS), fast-inv-sqrt on DVE
                    sq = pa.tile([128, 640], ATTN_DT, tag="sqv")
                    sq4 = sq[:].rearrange("p (t b x) -> p t b x", t=2, x=64)
                    nc.scalar.square(out=sq4, in_=rot4)
                    ss = pa.tile([128, 10], dt.float32, tag="ss")
                    nc.vector.tensor_reduce(
                        out=ss[:].rearrange("p (t b x) -> p t b x", t=2, x=1),
                        in_=sq4, op=OP.add, axis=mybir.AxisListType.X)
                    wv = pa.tile([128, 10], dt.float32, tag="wv")
                    nc.vector.tensor_scalar(
                        out=wv[:], in0=ss[:], scalar1=S_PRIME, scalar2=EPS,
                        op0=OP.mult, op1=OP.add)
                    yi = pa.tile([128, 10], dt.int32, tag="yi")
                    nc.vector.tensor_scalar(
                        out=yi[:], in0=wv[:].bitcast(dt.int32), scalar1=1,
                        scalar2=None, op0=OP.logical_shift_right)
                    nc.vector.tensor_tensor(
                        out=yi[:], in0=cst[:].to_broadcast((128, 10)), in1=yi[:],
                        op=OP.subtract)
                    y0 = yi[:].bitcast(dt.float32)
                    t1 = pa.tile([128, 10], dt.float32, tag="t1")
                    for _ in range(2):
                        nc.vector.tensor_tensor(out=t1[:], in0=y0, in1=y0, op=OP.mult)
                        nc.vector.tensor_tensor(out=t1[:], in0=t1[:], in1=wv[:],
                                                op=OP.mult)
                        nc.vector.scalar_tensor_tensor(
                            out=t1[:], in0=t1[:], scalar=-0.5,
                            in1=c15[:].to_broadcast((128, 10)),
                            op0=OP.mult, op1=OP.add)
                        nc.vector.tensor_tensor(out=y0, in0=y0, in1=t1[:], op=OP.mult)

                    qk_hat = pa.tile([128, 640], ATTN_DT, tag="qk_hat")
                    qk_hat4 = qk_hat[:].rearrange("p (t b x) -> p t b x", t=2, x=64)
                    rb = bcast_dim(
                        yi[:].bitcast(dt.float32).rearrange("p (t b) -> p t b", t=2),
                        3, 64)
                    nc.vector.tensor_tensor(out=qk_hat4, in0=rot4, in1=rb, op=OP.mult)

                    # transposes -> qT / kT
                    for tt in range(2):
                        s_tile = 2 * t2 + tt
                        tp = tp_ps.tile([128, 384], ATTN_DT, tag="tp")
                        for ft in range(2):
                            nc.tensor.transpose(
                                tp[:, 128 * ft:128 * (ft + 1)],
                                qk_hat[:, tt * 320 + 128 * ft:
                                       tt * 320 + 128 * (ft + 1)],
                                id_sb[:])
                        nc.tensor.transpose(
                            tp[0:64, 256:384],
                            qk_hat[:, tt * 320 + 256: tt * 320 + 320], id_sb[:])
                        nc.tensor.transpose(
                            tp[64:128, 256:384],
                            qk_hat[:, tt * 320 + 256: tt * 320 + 320], id_sb[:])
                        qdst = bass.AP(
                            tensor=qT_sb.tensor, offset=128 * s_tile,
                            ap=[[2 * S, 128], [S, 2], [1, 128]])
                        nc.vector.tensor_copy(
                            out=qdst,
                            in_=tp[:, 0:256].rearrange("p (b x) -> p b x", b=2))
                        nc.scalar.copy(
                            out=kT_sb[:, 128 * s_tile:128 * (s_tile + 1)],
                            in_=tp[:, 256:384])

            # ============ Phase B: attention + normalize + out proj =========
            with (
                tc.tile_pool(name="sc_ps", bufs=2, space="PSUM") as sc_ps,
                tc.tile_pool(name="acc_ps", bufs=2, space="PSUM") as acc_ps,
            ):
                def emit_attention(u):
                    segs = _segments(u)
                    ncols = _cols(u)
                    for pair in range(2):
                        exps = []
                        for hh in range(2):
                            sc = sc_ps.tile([128, 1280], dt.float32, tag="sc",
                                            name=f"sc_{u}_{pair}_{hh}")
                            for kt, col, n, q_off, _m in segs:
                                nc.tensor.matmul(
                                    sc[:, col:col + n],
                                    kT_sb[64 * hh:64 * (hh + 1),
                                          128 * kt:128 * (kt + 1)],
                                    qT_sb[64 * hh:64 * (hh + 1),
                                          S * pair + 256 * u + q_off:
                                          S * pair + 256 * u + q_off + n],
                                    start=True, stop=True,
                                    tile_position=(64 * hh, 0),
                                )
                            ex = pb.tile([128, 1280], ATTN_DT, tag="ex",
                                         name=f"ex_{u}_{pair}_{hh}")
                            nc.scalar.activation(out=ex[:, 0:ncols],
                                                 in_=sc[:, 0:ncols], func=AF.Exp)
                            for kt, col, n, q_off, m in segs:
                                if m is None:
                                    continue
                                kind, blk = m
                                blk_ap = ex[:, col + blk: col + blk + 128]
                                if kind == "LT":  # keep j < i
                                    nc.gpsimd.affine_select(
                                        out=blk_ap, in_=blk_ap,
                                        compare_op=OP.is_gt, fill=0.0, base=0,
                                        pattern=[[-1, 128]], channel_multiplier=1)
                                else:             # UT: keep j >= i
                                    nc.gpsimd.affine_select(
                                        out=blk_ap, in_=blk_ap,
                                        compare_op=OP.is_ge, fill=0.0, base=0,
                                        pattern=[[1, 128]], channel_multiplier=-1)
                            exps.append(ex)

                        for hh in range(2):
                            h = 2 * pair + hh
                            ex = exps[hh]
                            yt = acc_ps.tile([65, 256], dt.float32, tag="acc",
                                             name=f"yt_{u}_{h}")
                            last = len(segs) - 1
                            for i, (kt, col, n, q_off, _m) in enumerate(segs):
                                nc.tensor.matmul(
                                    yt[:, q_off:q_off + n],
                                    v_view[:, kt, :],
                                    ex[:, col:col + n],
                                    start=(i == 0), stop=(i == last),
                                )
                            nc.vector.tensor_copy(
                                out=yraw[h][:, 256 * u:256 * (u + 1)],
                                in_=yt[0:65, :])

                def emit_gather(u):
                    den32 = pbx.tile([32, 32], dt.float32, tag="den32",
                                     name=f"den32_{u}")
                    for h in range(4):
                        nc.sync.dma_start(
                            out=den32[8 * h:8 * (h + 1), :],
                            in_=yraw[h][64:65, 256 * u:256 * (u + 1)])
                    den32r = pbx.tile([32, 32], dt.float32r, tag="den32r",
                                      name=f"den32r_{u}")
                    with nc.allow_low_precision(reason="f32r bits == f32"):
                        nc.vector.reciprocal(out=den32r[:], in_=den32[:])
                    rrs = []
                    for h in range(4):
                        rr_h = pbx.tile([1, 256], dt.float32r, tag=f"rr{h}",
                                        name=f"rr_{u}_{h}")
                        nc.sync.dma_start(
                            out=rr_h[:],
                            in_=den32r[8 * h:8 * (h + 1), :])
                        rrs.append(rr_h)
                    setattr(emit_gather, f"rr_{u}", rrs)

                def emit_tail(u):
                    rrs = getattr(emit_gather, f"rr_{u}")
                    for h in range(4):
                        pair, hh = divmod(h, 2)
                        rb_ps = acc_ps.tile([64, 256], dt.float32, tag="acc",
                                            name=f"rb_{u}_{h}")
                        nc.tensor.matmul(rb_ps[:], ones64[:], rrs[h][:],
                                         start=True, stop=True)
                        if hh == 0:
                            nc.vector.tensor_tensor(
                                out=pairT[pair][0:64, 256 * u:256 * (u + 1)],
                                in0=yraw[h][0:64, 256 * u:256 * (u + 1)],
                                in1=rb_ps[:], op=OP.mult)
                        else:
                            stg = pbx.tile([64, 256], dt.float32r, tag="stg",
                                           name=f"stg_{u}_{h}")
                            nc.vector.tensor_tensor(
                                out=stg[:],
                                in0=yraw[h][0:64, 256 * u:256 * (u + 1)],
                                in1=rb_ps[:], op=OP.mult)
                            nc.sync.dma_start(
                                out=pairT[pair][64:128, 256 * u:256 * (u + 1)],
                                in_=stg[:])
                    for t in (2 * u, 2 * u + 1):
                        ot = pc.tile([128, 1024], dt.float32, tag="ot",
                                     name=f"ot_{t}")
                        for n in range(2):
                            op_ps = acc_ps.tile([128, 512], dt.float32,
                                                tag="acc", name=f"op_{t}_{n}")
                            for pair in range(2):
                                nc.tensor.matmul(
                                    op_ps[:],
                                    pairT[pair][:, 128 * t:128 * (t + 1)],
                                    wo_sb[pair][:, 512 * n:512 * (n + 1)],
                                    start=(pair == 0), stop=(pair == 1),
                                )
                            if (t + n) % 2 == 0:
                                nc.vector.tensor_copy(
                                    out=ot[:, 512 * n:512 * (n + 1)], in_=op_ps[:])
                            else:
                                nc.scalar.copy(
                                    out=ot[:, 512 * n:512 * (n + 1)], in_=op_ps[:])
                        nc.sync.dma_start(out=out_d[128 * t:128 * (t + 1), :],
                                          in_=ot[:])

                # software-pipelined emission: tail lags attention by one u
                for u in range(NU):
                    emit_attention(u)
                    emit_gather(u)
                    if u >= 1:
                        emit_tail(u - 1)
                emit_tail(NU - 1)

    nc.finalize()
    return nc


_CACHED_NC = None


def _get_nc():
    global _CACHED_NC
    if _CACHED_NC is None:
        _CACHED_NC = build_nc()
    return _CACHED_NC


def _prep_core(x, ve, cosf, sinf, Wq, Wk, Wv, Wo, Wg, b, g):
    xT = np.ascontiguousarray(x[b].T.astype(np.float32))
    wg_pad = np.zeros((D, 2), np.float32)
    wg_pad[:Wg.shape[0], 0] = Wg[:, g]
    w_all = np.concatenate(
        [Wq[:, 256 * g:256 * (g + 1)], Wk[:, 64 * g:64 * (g + 1)],
         Wv[:, 64 * g:64 * (g + 1)], wg_pad], axis=1).astype(np.float32)
    wo_g = np.ascontiguousarray(Wo[256 * g:256 * (g + 1), :].astype(np.float32))
    ve3 = (3.0 * ve[b, :, 64 * g:64 * (g + 1)]).astype(ml_dtypes.bfloat16)
    return {
        "xT": xT, "W": w_all, "Wo": wo_g, "ve3": ve3,
        "cosF": cosf, "sinFs": sinf,
        "ident": np.eye(128, dtype=ml_dtypes.bfloat16),
    }


def kernel(x, ve, cos, sin, Wq, Wk, Wv, Wo, Wg, window_size):
    assert int(window_size) == W
    x = np.asarray(x, np.float32)
    ve = np.asarray(ve, np.float32)
    cos2 = np.asarray(cos, np.float32).reshape(S, 32)
    sin2 = np.asarray(sin, np.float32).reshape(S, 32)
    a = np.float32(A_SCALE)
    cosf = (a * np.concatenate([cos2, cos2], axis=1)).astype(ml_dtypes.bfloat16)
    sinf = (a * np.concatenate([sin2, -sin2], axis=1)).astype(ml_dtypes.bfloat16)

    nc = _get_nc()
    in_maps = [
        _prep_core(x, ve, cosf, sinf,
                   np.asarray(Wq, np.float32), np.asarray(Wk, np.float32),
                   np.asarray(Wv, np.float32), np.asarray(Wo, np.float32),
                   np.asarray(Wg, np.float32), c // 4, c % 4)
        for c in range(NCORES)
    ]
    res = run_bass_kernel_spmd(nc, in_maps, core_ids=list(range(NCORES)))
    out = np.zeros((B, S, D), np.float32)
    for c in range(NCORES):
        out[c // 4] += res.results[c]["out"]
    return out
